# revision 32
# baseline (speedup 1.0000x reference)
"""CTPNet Trainium2 kernel: 8-way data-parallel over the batch dim.

Net (per reference):
    h1 = relu(x @ W1.T + b1)            x:[B,G]  W1:[H1,G]   -> [B,512]
    h2 = relu(h1 @ W2.T + b2)           W2:[H2,H1]           -> [B,256]
    a  = relu(einsum(bh,pha->bpa) + b3) W3:[P,H2,A]          -> [B,P,A]
    out= einsum(bpa,pa->bp) + b4        W4:[P,A]             -> [B,P]

B,G,H1,H2,P,A = 8192, 20000, 512, 256, 128, 64

Strategy: shard x rows 8 ways (1024 rows/core), replicate weights.
All on-chip tensors live in the "transposed" layout (feature dim on
partitions, batch on the free axis) so every layer is a plain chain of
TensorE matmuls with K (contraction) on the partition dim:

    h1T[512,1024]  = W1T.T-tiles @ xT-tiles          (157 K-tiles over G)
    h2T[256,1024]  = W2T-tiles @ h1T
    aT  [8192,1024] = W3f-tiles @ h2T  (heads flattened to [H2, P*A])
    outT[128,1024] = W4bd-tiles @ aT   (block-diag W4 does the A-reduction)

Host pre-transposes weights/x so every DMA is contiguous; host
re-assembles out = concat(outT_c.T).  Biases are folded into the
PSUM->SBUF eviction (Relu with per-partition bias, split across
ScalarE/VectorE).  Default compute dtype is bf16 (fro rel err ~4.4e-3
vs the f32 reference); CTP_MODE=f32r selects float32r end-to-end
(~2.9e-4, ~15% slower, DMA-bound).
"""

import os
import numpy as np

B, G, H1, H2, P, A = 8192, 20000, 512, 256, 128, 64
NCORES = 8
BC = B // NCORES            # 1024 batch rows per core
NBC = 2                     # b-chunks per core
BW = BC // NBC              # 512 (one PSUM bank / max fp32 moving free dim)
KT = 128
NKP = 157                   # K-tiles over G (156 full + one K=32 tail)
GP = NKP * KT               # 20096 (zero-padded from 20000)
KG = 4                      # max k-tiles per DMA group
# ramp-up schedule: small first chunks so the PE starts ~10us sooner
_sizes = [1, 1, 2] + [KG] * ((NKP - 4) // KG) + (
    [NKP - 4 - KG * ((NKP - 4) // KG)] if (NKP - 4) % KG else []
)
KGROUPS = []
_k = 0
for _s in _sizes:
    KGROUPS.append((_k, _s))
    _k += _s
assert _k == NKP
PA = P * A                  # 8192 flattened head outputs
NM3 = PA // 128             # 64 pa-chunks
NM1 = H1 // 128             # 4
NK2 = H1 // 128             # 4
NM2 = H2 // 128             # 2
NK3 = H2 // 128             # 2

_CACHE = {}


def _build(mode: str):
    """Build + compile the per-core Bass graph. mode: 'f32r' | 'f32' | 'bf16'."""
    import concourse.bacc as bacc
    import concourse.tile as tile
    import concourse.mybir as mybir

    FP = mybir.dt.float32
    # ST: storage dtype of streamed/intermediate SBUF tiles (and big DRAM ins)
    ST = mybir.dt.bfloat16 if mode.startswith("bf16") else (
        mybir.dt.float32r if mode == "f32r" else mybir.dt.float32
    )
    DEEP = 6 if mode == "bf16x" else 4

    def mc(ap):
        return ap

    nc = bacc.Bacc(
        "TRN2", target_bir_lowering=False, debug=False, num_devices=NCORES
    )

    # k-tile-interleaved layouts: col block ki holds k-tile ki
    # xTi[p, ki*BC + j]  = x_core[j, ki*128 + p]   (zero-padded past G)
    # w1ti[p, ki*H1 + h] = W1[h, ki*128 + p]
    xT_d = nc.dram_tensor("xTi", [128, NKP * BC], ST, kind="ExternalInput")
    w1t_d = nc.dram_tensor("w1ti", [128, NKP * H1], ST, kind="ExternalInput")
    b1r_d = nc.dram_tensor("b1r", [128, NM1], FP, kind="ExternalInput")
    w2t_d = nc.dram_tensor("w2t", [H1, H2], ST, kind="ExternalInput")
    b2r_d = nc.dram_tensor("b2r", [128, NM2], FP, kind="ExternalInput")
    w3f_d = nc.dram_tensor("w3f", [H2, PA], ST, kind="ExternalInput")
    b3r_d = nc.dram_tensor("b3r", [128, NM3], FP, kind="ExternalInput")
    w4bd_d = nc.dram_tensor("w4bd", [PA, P], ST, kind="ExternalInput")
    b4r_d = nc.dram_tensor("b4r", [128, 1], FP, kind="ExternalInput")
    out_d = nc.dram_tensor("out", [P, BC], FP, kind="ExternalOutput")

    Relu = mybir.ActivationFunctionType.Relu

    with tile.TileContext(nc) as tc:
        with (
            tc.tile_pool(name="const", bufs=1) as const,
            tc.tile_pool(name="h1", bufs=1) as h1pool,
            tc.tile_pool(name="h2", bufs=1) as h2pool,
            tc.tile_pool(name="osb", bufs=1) as opool,
            tc.tile_pool(name="xp", bufs=6 if mode.startswith("bf16") else 4) as xpool,
            tc.tile_pool(name="w1p", bufs=6 if mode.startswith("bf16") else 4) as w1pool,
        ):
            b1t = const.tile([128, NM1], FP)
            nc.scalar.dma_start(b1t[:], b1r_d[:])
            b2t = const.tile([128, NM2], FP)
            nc.scalar.dma_start(b2t[:], b2r_d[:])
            b3t = const.tile([128, NM3], FP)
            nc.scalar.dma_start(b3t[:], b3r_d[:])
            b4t = const.tile([128, 1], FP)
            nc.scalar.dma_start(b4t[:], b4r_d[:])

            # h1T as [128, m(4) x b(1024)]; col m*BC + j holds h1[m*128+p, j]
            h1a = h1pool.tile([128, NM1 * BC], ST)
            h2a = h2pool.tile([128, NM2 * BC], ST)
            outsb = opool.tile([128, BC], FP)

            # ---- L1 (+ L2 interleaved into the L1 tail) ----
            # L1: h1T = relu(W1T.T @ xT + b1), K over G
            # L2: h2T = relu(W2T.T @ h1T + b2), K over H1
            # The last two k-groups finish the 8 PSUM banks one at a time;
            # each bank's eviction and the dependent L2 matmuls slot in
            # between the remaining L1 matmuls so the PE never idles (and
            # HAM never rethrottles) across the phase transition.
            def evict_l1(m, b):
                c0 = m * BC + b * BW
                if (m * NBC + b) % 2 == 0:
                    nc.scalar.activation(
                        h1a[:, c0 : c0 + BW],
                        pst[m * NBC + b][:, :],
                        Relu,
                        bias=b1t[:, m : m + 1],
                    )
                else:
                    nc.vector.tensor_scalar(
                        h1a[:, c0 : c0 + BW],
                        pst[m * NBC + b][:, :],
                        b1t[:, m : m + 1],
                        0.0,
                        op0=mybir.AluOpType.add,
                        op1=mybir.AluOpType.max,
                    )

            def l2_mms(ki):
                for m in range(NM2):
                    for b in range(NBC):
                        nc.tensor.matmul(
                            pst2[m * NBC + b][:, :],
                            mc(w2tls[ki][:, m * 128 : (m + 1) * 128]),
                            mc(h1a[:, ki * BC + b * BW : ki * BC + b * BW + BW]),
                            start=(ki == 0),
                            stop=(ki == NK2 - 1),
                        )

            with (
                tc.tile_pool(name="ps1", bufs=1, space="PSUM") as ps1,
                tc.tile_pool(name="w2p", bufs=NK2) as w2pool,
            ):
                pst = [
                    ps1.tile([128, BW], FP, name=f"pst{i}", tag=f"pst{i}")
                    for i in range(NM1 * NBC)
                ]
                # L2 accumulators reuse the first four L1 banks (same tags);
                # each waits for its L1 predecessor's eviction to release it.
                pst2 = [
                    ps1.tile([128, BW], FP, name=f"pst2_{i}", tag=f"pst{i}")
                    for i in range(NM2 * NBC)
                ]
                w2tls = []
                for ki in range(NK2):
                    w2t = w2pool.tile([128, H2], ST, name=f"w2t{ki}", tag=f"w2t{ki}")
                    nc.scalar.dma_start(w2t[:], w2t_d[ki * 128 : (ki + 1) * 128, :])
                    w2tls.append(w2t)

                tiles = {}
                for (k0, gsz) in KGROUPS[:-2]:
                    wt = w1pool.tile([128, KG * H1], ST)
                    nc.sync.dma_start(
                        wt[:, : gsz * H1], w1t_d[:, k0 * H1 : (k0 + gsz) * H1]
                    )
                    xt = xpool.tile([128, KG * BC], ST)
                    nc.sync.dma_start(
                        xt[:, : gsz * BC], xT_d[:, k0 * BC : (k0 + gsz) * BC]
                    )
                    # k-contiguous per PSUM bank: bank switches every gsz MMs
                    # (not every MM) to avoid psum-queue depth-cycling stalls
                    for m in range(NM1):
                        for b in range(NBC):
                            for kk in range(gsz):
                                ki = k0 + kk
                                nc.tensor.matmul(
                                    pst[m * NBC + b][:, :],
                                    mc(wt[:, kk * H1 + m * 128 : kk * H1 + (m + 1) * 128]),
                                    mc(xt[:, kk * BC + b * BW : kk * BC + (b + 1) * BW]),
                                    start=(ki == 0),
                                    stop=False,
                                )
                # tail: per-bank finish + eviction + L2 interleave
                for (k0, gsz) in KGROUPS[-2:]:
                    wt = w1pool.tile([128, KG * H1], ST)
                    nc.sync.dma_start(
                        wt[:, : gsz * H1], w1t_d[:, k0 * H1 : (k0 + gsz) * H1]
                    )
                    xt = xpool.tile([128, KG * BC], ST)
                    nc.sync.dma_start(
                        xt[:, : gsz * BC], xT_d[:, k0 * BC : (k0 + gsz) * BC]
                    )
                    tiles[k0] = (wt, xt)
                for m in range(NM1):
                    for b in range(NBC):
                        for (k0, gsz) in KGROUPS[-2:]:
                            wt, xt = tiles[k0]
                            for kk in range(gsz):
                                ki = k0 + kk
                                nc.tensor.matmul(
                                    pst[m * NBC + b][:, :],
                                    mc(wt[:, kk * H1 + m * 128 : kk * H1 + (m + 1) * 128]),
                                    mc(xt[:, kk * BC + b * BW : kk * BC + (b + 1) * BW]),
                                    start=False,
                                    stop=(ki == NKP - 1),
                                )
                        evict_l1(m, b)
                    # 2 banks freed per m; L2 needs 4 -> start at m>=1
                    if m == 1:
                        l2_mms(0)
                        l2_mms(1)
                    elif m == 2:
                        l2_mms(2)
                    elif m == 3:
                        l2_mms(3)
                for m in range(NM2):
                    for b in range(NBC):
                        c0 = m * BC + b * BW
                        if (m * NBC + b) % 2 == 0:
                            nc.scalar.activation(
                                h2a[:, c0 : c0 + BW],
                                pst2[m * NBC + b][:, :],
                                Relu,
                                bias=b2t[:, m : m + 1],
                            )
                        else:
                            nc.vector.tensor_scalar(
                                h2a[:, c0 : c0 + BW],
                                pst2[m * NBC + b][:, :],
                                b2t[:, m : m + 1],
                                0.0,
                                op0=mybir.AluOpType.add,
                                op1=mybir.AluOpType.max,
                            )

            # ---- L3+L4: aT chunks then block-diag W4 reduction ----
            with (
                tc.tile_pool(name="w3p", bufs=DEEP) as w3pool,
                tc.tile_pool(name="w4p", bufs=DEEP) as w4pool,
                tc.tile_pool(name="ap", bufs=DEEP) as apool,
                tc.tile_pool(name="ps4", bufs=1, space="PSUM") as ps4,
                tc.tile_pool(name="ps3", bufs=4, space="PSUM") as ps3pool,
            ):
                po = [ps4.tile([128, BW], FP, name=f"po{i}", tag=f"po{i}") for i in range(NBC)]
                # software-pipelined: L4 accumulation for step mi-1 is emitted
                # between step mi's L3 matmuls so the PSUM->SBUF eviction
                # latency never blocks the PE stream.
                pend = []  # (mi, b, w4t, at) awaiting their L4 matmul

                def flush_l4():
                    for (pmi, pb, pw4t, pat) in pend:
                        nc.tensor.matmul(
                            po[pb][:, :],
                            mc(pw4t[:, :]),
                            mc(pat[:, :]),
                            start=(pmi == 0),
                            stop=(pmi == NM3 - 1),
                        )
                    pend.clear()

                for mi in range(NM3):
                    w3t = w3pool.tile([128, H2], ST)
                    for k in range(NK3):
                        nc.sync.dma_start(
                            w3t[:, k * 128 : (k + 1) * 128],
                            w3f_d[k * 128 : (k + 1) * 128, mi * 128 : (mi + 1) * 128],
                        )
                    w4t = w4pool.tile([128, 128], ST)
                    nc.sync.dma_start(w4t[:], w4bd_d[mi * 128 : (mi + 1) * 128, :])
                    mypend = []
                    for b in range(NBC):
                        ps3 = ps3pool.tile([128, BW], FP)
                        for k in range(NK3):
                            nc.tensor.matmul(
                                ps3[:, :],
                                mc(w3t[:, k * 128 : (k + 1) * 128]),
                                mc(h2a[:, k * BC + b * BW : k * BC + b * BW + BW]),
                                start=(k == 0),
                                stop=(k == NK3 - 1),
                            )
                        if b == NBC - 1:
                            flush_l4()  # previous mi's L4 after this mi's L3 MMs
                        at = apool.tile([128, BW], ST)
                        if (mi * NBC + b) % 5 < 3:
                            nc.scalar.activation(
                                at[:, :], ps3[:, :], Relu, bias=b3t[:, mi : mi + 1]
                            )
                        else:
                            # relu(x + b3) on VectorE: (x add b3) max 0
                            nc.vector.tensor_scalar(
                                at[:, :],
                                ps3[:, :],
                                b3t[:, mi : mi + 1],
                                0.0,
                                op0=mybir.AluOpType.add,
                                op1=mybir.AluOpType.max,
                            )
                        mypend.append((mi, b, w4t, at))
                    pend.extend(mypend)
                flush_l4()
                for b in range(NBC):
                    nc.vector.tensor_scalar_add(
                        outsb[:, b * BW : (b + 1) * BW], po[b][:, :], b4t[:, 0:1]
                    )
            nc.sync.dma_start(out_d[:, :], outsb[:, :])

    nc.compile()
    return nc


def _get_nc(mode: str):
    if mode not in _CACHE:
        _CACHE[mode] = _build(mode)
    return _CACHE[mode]


def _interleave_k(mat_gp: np.ndarray) -> np.ndarray:
    """[GP, F] -> [128, NKP*F] with col block ki = k-tile ki."""
    f = mat_gp.shape[1]
    return np.ascontiguousarray(
        mat_gp.reshape(NKP, 128, f).transpose(1, 0, 2).reshape(128, NKP * f)
    )


def _prep_inputs(x, W1, b1, W2, b2, W3, b3, W4, b4, mode="f32r"):
    f = np.float32
    if mode.startswith("bf16"):
        import ml_dtypes

        st = np.dtype(ml_dtypes.bfloat16)
    else:
        st = np.dtype(np.float32)
    ac = np.ascontiguousarray

    def cst(a):
        return a if a.dtype == st else a.astype(st)

    x = np.asarray(x, f)
    xTp = np.zeros((GP, B), st)
    np.copyto(xTp[:G], cst(x.T))                               # [GP, B]
    w1tp = np.zeros((GP, H1), st)
    np.copyto(w1tp[:G], cst(np.asarray(W1, f).T))
    w1ti = _interleave_k(w1tp)                                 # [128, NKP*H1]
    b1r = ac(np.asarray(b1, f).reshape(NM1, 128).T)            # [128, 4]
    w2t = ac(cst(np.asarray(W2, f).T))                         # [H1, H2]
    b2r = ac(np.asarray(b2, f).reshape(NM2, 128).T)            # [128, 2]
    w3f = ac(cst(np.asarray(W3, f).transpose(1, 0, 2).reshape(H2, PA)))
    b3r = ac(np.asarray(b3, f).reshape(PA).reshape(NM3, 128).T)  # [128, 64]
    w4bd = np.zeros((PA, P), st)
    w4bd[np.arange(PA), np.arange(PA) // A] = cst(np.asarray(W4, f).reshape(PA))
    b4r = ac(np.asarray(b4, f).reshape(128, 1))

    shared = {
        "w1ti": w1ti, "b1r": b1r, "w2t": w2t, "b2r": b2r,
        "w3f": w3f, "b3r": b3r, "w4bd": w4bd, "b4r": b4r,
    }
    in_maps = []
    for c in range(NCORES):
        m = {"xTi": _interleave_k(xTp[:, c * BC : (c + 1) * BC])}
        m.update(shared)
        in_maps.append(m)
    return in_maps


def run_with_results(inputs: dict, trace: bool = False, mode: str | None = None):
    """Returns (full_output [B, P] float32, BassKernelResults)."""
    from concourse.bass_utils import run_bass_kernel_spmd

    if mode is None:
        mode = os.environ.get("CTP_MODE", "bf16x")
    nc = _get_nc(mode)
    in_maps = _prep_inputs(**inputs, mode=mode)
    res = run_bass_kernel_spmd(
        nc, in_maps, core_ids=list(range(NCORES)), trace=trace
    )
    out = np.empty((B, P), np.float32)
    for c in range(NCORES):
        out[c * BC : (c + 1) * BC, :] = res.results[c]["out"].T
    return out, res


def kernel(**inputs) -> np.ndarray:
    out, _ = run_with_results(inputs, trace=False)
    return out


# revision 33
# speedup vs baseline: 1.0128x; 1.0128x over previous
"""CTPNet Trainium2 kernel: 8-way data-parallel over the batch dim.

Net (per reference):
    h1 = relu(x @ W1.T + b1)            x:[B,G]  W1:[H1,G]   -> [B,512]
    h2 = relu(h1 @ W2.T + b2)           W2:[H2,H1]           -> [B,256]
    a  = relu(einsum(bh,pha->bpa) + b3) W3:[P,H2,A]          -> [B,P,A]
    out= einsum(bpa,pa->bp) + b4        W4:[P,A]             -> [B,P]

B,G,H1,H2,P,A = 8192, 20000, 512, 256, 128, 64

Strategy: shard x rows 8 ways (1024 rows/core), replicate weights.
All on-chip tensors live in the "transposed" layout (feature dim on
partitions, batch on the free axis) so every layer is a plain chain of
TensorE matmuls with K (contraction) on the partition dim:

    h1T[512,1024]  = W1T.T-tiles @ xT-tiles          (157 K-tiles over G)
    h2T[256,1024]  = W2T-tiles @ h1T
    aT  [8192,1024] = W3f-tiles @ h2T  (heads flattened to [H2, P*A])
    outT[128,1024] = W4bd-tiles @ aT   (block-diag W4 does the A-reduction)

Host pre-transposes weights/x so every DMA is contiguous; host
re-assembles out = concat(outT_c.T).  Biases are folded into the
PSUM->SBUF eviction (Relu with per-partition bias, split across
ScalarE/VectorE).  Default compute dtype is bf16 (fro rel err ~4.4e-3
vs the f32 reference); CTP_MODE=f32r selects float32r end-to-end
(~2.9e-4, ~15% slower, DMA-bound).
"""

import os
import numpy as np

B, G, H1, H2, P, A = 8192, 20000, 512, 256, 128, 64
NCORES = 8
BC = B // NCORES            # 1024 batch rows per core
NBC = 2                     # b-chunks per core
BW = BC // NBC              # 512 (one PSUM bank / max fp32 moving free dim)
KT = 128
NKP = 157                   # K-tiles over G (156 full + one K=32 tail)
GP = NKP * KT               # 20096 (zero-padded from 20000)
KG = 4                      # max k-tiles per DMA group
# ramp-up schedule: small first chunks so the PE starts ~10us sooner
_sizes = [1, 1, 2] + [KG] * ((NKP - 4) // KG) + (
    [NKP - 4 - KG * ((NKP - 4) // KG)] if (NKP - 4) % KG else []
)
KGROUPS = []
_k = 0
for _s in _sizes:
    KGROUPS.append((_k, _s))
    _k += _s
assert _k == NKP
PA = P * A                  # 8192 flattened head outputs
NM3 = PA // 128             # 64 pa-chunks
NM1 = H1 // 128             # 4
NK2 = H1 // 128             # 4
NM2 = H2 // 128             # 2
NK3 = H2 // 128             # 2

_CACHE = {}


def _build(mode: str):
    """Build + compile the per-core Bass graph. mode: 'f32r' | 'f32' | 'bf16'."""
    import concourse.bacc as bacc
    import concourse.tile as tile
    import concourse.mybir as mybir

    FP = mybir.dt.float32
    # ST: storage dtype of streamed/intermediate SBUF tiles (and big DRAM ins)
    ST = mybir.dt.bfloat16 if mode.startswith("bf16") else (
        mybir.dt.float32r if mode == "f32r" else mybir.dt.float32
    )
    DEEP = 6 if mode == "bf16x" else 4

    def mc(ap):
        return ap

    nc = bacc.Bacc(
        "TRN2", target_bir_lowering=False, debug=False, num_devices=NCORES
    )

    # k-tile-interleaved layouts: col block ki holds k-tile ki
    # xTi[p, ki*BC + j]  = x_core[j, ki*128 + p]   (zero-padded past G)
    # w1ti[p, ki*H1 + h] = W1[h, ki*128 + p]
    xT_d = nc.dram_tensor("xTi", [128, NKP * BC], ST, kind="ExternalInput")
    w1t_d = nc.dram_tensor("w1ti", [128, NKP * H1], ST, kind="ExternalInput")
    b1r_d = nc.dram_tensor("b1r", [128, NM1], FP, kind="ExternalInput")
    w2t_d = nc.dram_tensor("w2t", [H1, H2], ST, kind="ExternalInput")
    b2r_d = nc.dram_tensor("b2r", [128, NM2], FP, kind="ExternalInput")
    w3f_d = nc.dram_tensor("w3f", [H2, PA], ST, kind="ExternalInput")
    b3r_d = nc.dram_tensor("b3r", [128, NM3], FP, kind="ExternalInput")
    w4bd_d = nc.dram_tensor("w4bd", [PA, P], ST, kind="ExternalInput")
    b4r_d = nc.dram_tensor("b4r", [128, 1], FP, kind="ExternalInput")
    out_d = nc.dram_tensor("out", [P, BC], FP, kind="ExternalOutput")

    Relu = mybir.ActivationFunctionType.Relu

    with tile.TileContext(nc) as tc:
        with (
            tc.tile_pool(name="const", bufs=1) as const,
            tc.tile_pool(name="h1", bufs=1) as h1pool,
            tc.tile_pool(name="h2", bufs=1) as h2pool,
            tc.tile_pool(name="osb", bufs=1) as opool,
            tc.tile_pool(name="xp", bufs=6 if mode.startswith("bf16") else 4) as xpool,
            tc.tile_pool(name="w1p", bufs=6 if mode.startswith("bf16") else 4) as w1pool,
        ):
            b1t = const.tile([128, NM1], FP)
            nc.scalar.dma_start(b1t[:], b1r_d[:])
            b2t = const.tile([128, NM2], FP)
            nc.scalar.dma_start(b2t[:], b2r_d[:])
            b3t = const.tile([128, NM3], FP)
            nc.scalar.dma_start(b3t[:], b3r_d[:])
            b4t = const.tile([128, 1], FP)
            nc.scalar.dma_start(b4t[:], b4r_d[:])

            # h1T as [128, m(4) x b(1024)]; col m*BC + j holds h1[m*128+p, j]
            h1a = h1pool.tile([128, NM1 * BC], ST)
            h2a = h2pool.tile([128, NM2 * BC], ST)
            outsb = opool.tile([128, BC], FP)

            # ---- L1: h1T = relu(W1T.T @ xT + b1), K over G ----
            with tc.tile_pool(name="ps1", bufs=1, space="PSUM") as ps1:
                pst = [
                    ps1.tile([128, BW], FP, name=f"pst{i}", tag=f"pst{i}")
                    for i in range(NM1 * NBC)
                ]
                for (k0, gsz) in KGROUPS:
                    wt = w1pool.tile([128, KG * H1], ST)
                    nc.sync.dma_start(
                        wt[:, : gsz * H1], w1t_d[:, k0 * H1 : (k0 + gsz) * H1]
                    )
                    xt = xpool.tile([128, KG * BC], ST)
                    nc.sync.dma_start(
                        xt[:, : gsz * BC], xT_d[:, k0 * BC : (k0 + gsz) * BC]
                    )
                    # k-contiguous per PSUM bank: bank switches every gsz MMs
                    # (not every MM) to avoid psum-queue depth-cycling stalls
                    for m in range(NM1):
                        for b in range(NBC):
                            for kk in range(gsz):
                                ki = k0 + kk
                                nc.tensor.matmul(
                                    pst[m * NBC + b][:, :],
                                    mc(wt[:, kk * H1 + m * 128 : kk * H1 + (m + 1) * 128]),
                                    mc(xt[:, kk * BC + b * BW : kk * BC + (b + 1) * BW]),
                                    start=(ki == 0),
                                    stop=(ki == NKP - 1),
                                )
                for m in range(NM1):
                    for b in range(NBC):
                        c0 = m * BC + b * BW
                        if (m * NBC + b) % 2 == 0:
                            nc.scalar.activation(
                                h1a[:, c0 : c0 + BW],
                                pst[m * NBC + b][:, :],
                                Relu,
                                bias=b1t[:, m : m + 1],
                            )
                        else:
                            nc.vector.tensor_scalar(
                                h1a[:, c0 : c0 + BW],
                                pst[m * NBC + b][:, :],
                                b1t[:, m : m + 1],
                                0.0,
                                op0=mybir.AluOpType.add,
                                op1=mybir.AluOpType.max,
                            )

            # ---- L2: h2T = relu(W2T.T @ h1T + b2), K over H1 ----
            with (
                tc.tile_pool(name="w2p", bufs=NK2) as w2pool,
                tc.tile_pool(name="ps2", bufs=1, space="PSUM") as ps2,
            ):
                pst2 = [
                    ps2.tile([128, BW], FP, name=f"pst2_{i}", tag=f"pst2_{i}")
                    for i in range(NM2 * NBC)
                ]
                for ki in range(NK2):
                    w2t = w2pool.tile([128, H2], ST)
                    nc.scalar.dma_start(w2t[:], w2t_d[ki * 128 : (ki + 1) * 128, :])
                    for m in range(NM2):
                        for b in range(NBC):
                            nc.tensor.matmul(
                                pst2[m * NBC + b][:, :],
                                mc(w2t[:, m * 128 : (m + 1) * 128]),
                                mc(h1a[:, ki * BC + b * BW : ki * BC + b * BW + BW]),
                                start=(ki == 0),
                                stop=(ki == NK2 - 1),
                            )
                for m in range(NM2):
                    for b in range(NBC):
                        c0 = m * BC + b * BW
                        if (m * NBC + b) % 2 == 0:
                            nc.scalar.activation(
                                h2a[:, c0 : c0 + BW],
                                pst2[m * NBC + b][:, :],
                                Relu,
                                bias=b2t[:, m : m + 1],
                            )
                        else:
                            nc.vector.tensor_scalar(
                                h2a[:, c0 : c0 + BW],
                                pst2[m * NBC + b][:, :],
                                b2t[:, m : m + 1],
                                0.0,
                                op0=mybir.AluOpType.add,
                                op1=mybir.AluOpType.max,
                            )

            # ---- L3+L4: aT chunks then block-diag W4 reduction ----
            with (
                tc.tile_pool(name="w3p", bufs=DEEP) as w3pool,
                tc.tile_pool(name="w4p", bufs=DEEP) as w4pool,
                tc.tile_pool(name="ap", bufs=DEEP) as apool,
                tc.tile_pool(name="ps4", bufs=1, space="PSUM") as ps4,
                tc.tile_pool(name="ps3", bufs=4, space="PSUM") as ps3pool,
            ):
                po = [ps4.tile([128, BW], FP, name=f"po{i}", tag=f"po{i}") for i in range(NBC)]
                # software-pipelined: L4 accumulation for step mi-1 is emitted
                # between step mi's L3 matmuls so the PSUM->SBUF eviction
                # latency never blocks the PE stream.
                pend = []  # (mi, b, w4t, at) awaiting their L4 matmul

                def flush_l4():
                    for (pmi, pb, pw4t, pat) in pend:
                        nc.tensor.matmul(
                            po[pb][:, :],
                            mc(pw4t[:, :]),
                            mc(pat[:, :]),
                            start=(pmi == 0),
                            stop=(pmi == NM3 - 1),
                        )
                    pend.clear()

                for mi in range(NM3):
                    w3t = w3pool.tile([128, H2], ST)
                    for k in range(NK3):
                        nc.sync.dma_start(
                            w3t[:, k * 128 : (k + 1) * 128],
                            w3f_d[k * 128 : (k + 1) * 128, mi * 128 : (mi + 1) * 128],
                        )
                    w4t = w4pool.tile([128, 128], ST)
                    nc.sync.dma_start(w4t[:], w4bd_d[mi * 128 : (mi + 1) * 128, :])
                    mypend = []
                    for b in range(NBC):
                        ps3 = ps3pool.tile([128, BW], FP)
                        for k in range(NK3):
                            nc.tensor.matmul(
                                ps3[:, :],
                                mc(w3t[:, k * 128 : (k + 1) * 128]),
                                mc(h2a[:, k * BC + b * BW : k * BC + b * BW + BW]),
                                start=(k == 0),
                                stop=(k == NK3 - 1),
                            )
                        if b == NBC - 1:
                            flush_l4()  # previous mi's L4 after this mi's L3 MMs
                        at = apool.tile([128, BW], ST)
                        if (mi * NBC + b) % 5 < 3:
                            nc.scalar.activation(
                                at[:, :], ps3[:, :], Relu, bias=b3t[:, mi : mi + 1]
                            )
                        else:
                            # relu(x + b3) on VectorE: (x add b3) max 0
                            nc.vector.tensor_scalar(
                                at[:, :],
                                ps3[:, :],
                                b3t[:, mi : mi + 1],
                                0.0,
                                op0=mybir.AluOpType.add,
                                op1=mybir.AluOpType.max,
                            )
                        mypend.append((mi, b, w4t, at))
                    pend.extend(mypend)
                flush_l4()
                for b in range(NBC):
                    nc.vector.tensor_scalar_add(
                        outsb[:, b * BW : (b + 1) * BW], po[b][:, :], b4t[:, 0:1]
                    )
            nc.sync.dma_start(out_d[:, :], outsb[:, :])

    nc.compile()
    return nc


def _get_nc(mode: str):
    if mode not in _CACHE:
        _CACHE[mode] = _build(mode)
    return _CACHE[mode]


def _interleave_k(mat_gp: np.ndarray) -> np.ndarray:
    """[GP, F] -> [128, NKP*F] with col block ki = k-tile ki."""
    f = mat_gp.shape[1]
    return np.ascontiguousarray(
        mat_gp.reshape(NKP, 128, f).transpose(1, 0, 2).reshape(128, NKP * f)
    )


def _prep_inputs(x, W1, b1, W2, b2, W3, b3, W4, b4, mode="f32r"):
    f = np.float32
    if mode.startswith("bf16"):
        import ml_dtypes

        st = np.dtype(ml_dtypes.bfloat16)
    else:
        st = np.dtype(np.float32)
    ac = np.ascontiguousarray

    def cst(a):
        return a if a.dtype == st else a.astype(st)

    x = np.asarray(x, f)
    xTp = np.zeros((GP, B), st)
    np.copyto(xTp[:G], cst(x.T))                               # [GP, B]
    w1tp = np.zeros((GP, H1), st)
    np.copyto(w1tp[:G], cst(np.asarray(W1, f).T))
    w1ti = _interleave_k(w1tp)                                 # [128, NKP*H1]
    b1r = ac(np.asarray(b1, f).reshape(NM1, 128).T)            # [128, 4]
    w2t = ac(cst(np.asarray(W2, f).T))                         # [H1, H2]
    b2r = ac(np.asarray(b2, f).reshape(NM2, 128).T)            # [128, 2]
    w3f = ac(cst(np.asarray(W3, f).transpose(1, 0, 2).reshape(H2, PA)))
    b3r = ac(np.asarray(b3, f).reshape(PA).reshape(NM3, 128).T)  # [128, 64]
    w4bd = np.zeros((PA, P), st)
    w4bd[np.arange(PA), np.arange(PA) // A] = cst(np.asarray(W4, f).reshape(PA))
    b4r = ac(np.asarray(b4, f).reshape(128, 1))

    shared = {
        "w1ti": w1ti, "b1r": b1r, "w2t": w2t, "b2r": b2r,
        "w3f": w3f, "b3r": b3r, "w4bd": w4bd, "b4r": b4r,
    }
    in_maps = []
    for c in range(NCORES):
        m = {"xTi": _interleave_k(xTp[:, c * BC : (c + 1) * BC])}
        m.update(shared)
        in_maps.append(m)
    return in_maps


def run_with_results(inputs: dict, trace: bool = False, mode: str | None = None):
    """Returns (full_output [B, P] float32, BassKernelResults)."""
    from concourse.bass_utils import run_bass_kernel_spmd

    if mode is None:
        mode = os.environ.get("CTP_MODE", "bf16x")
    nc = _get_nc(mode)
    in_maps = _prep_inputs(**inputs, mode=mode)
    res = run_bass_kernel_spmd(
        nc, in_maps, core_ids=list(range(NCORES)), trace=trace
    )
    out = np.empty((B, P), np.float32)
    for c in range(NCORES):
        out[c * BC : (c + 1) * BC, :] = res.results[c]["out"].T
    return out, res


def kernel(**inputs) -> np.ndarray:
    out, _ = run_with_results(inputs, trace=False)
    return out


# revision 34
# speedup vs baseline: 1.0159x; 1.0031x over previous
"""CTPNet Trainium2 kernel: 8-way data-parallel over the batch dim.

Net (per reference):
    h1 = relu(x @ W1.T + b1)            x:[B,G]  W1:[H1,G]   -> [B,512]
    h2 = relu(h1 @ W2.T + b2)           W2:[H2,H1]           -> [B,256]
    a  = relu(einsum(bh,pha->bpa) + b3) W3:[P,H2,A]          -> [B,P,A]
    out= einsum(bpa,pa->bp) + b4        W4:[P,A]             -> [B,P]

B,G,H1,H2,P,A = 8192, 20000, 512, 256, 128, 64

Strategy: shard x rows 8 ways (1024 rows/core), replicate weights.
All on-chip tensors live in the "transposed" layout (feature dim on
partitions, batch on the free axis) so every layer is a plain chain of
TensorE matmuls with K (contraction) on the partition dim:

    h1T[512,1024]  = W1T.T-tiles @ xT-tiles          (157 K-tiles over G)
    h2T[256,1024]  = W2T-tiles @ h1T
    aT  [8192,1024] = W3f-tiles @ h2T  (heads flattened to [H2, P*A])
    outT[128,1024] = W4bd-tiles @ aT   (block-diag W4 does the A-reduction)

Host pre-transposes weights/x so every DMA is contiguous; host
re-assembles out = concat(outT_c.T).  Biases are folded into the
PSUM->SBUF eviction (Relu with per-partition bias, split across
ScalarE/VectorE).  Default compute dtype is bf16 (fro rel err ~4.4e-3
vs the f32 reference); CTP_MODE=f32r selects float32r end-to-end
(~2.9e-4, ~15% slower, DMA-bound).
"""

import os
import numpy as np

B, G, H1, H2, P, A = 8192, 20000, 512, 256, 128, 64
NCORES = 8
BC = B // NCORES            # 1024 batch rows per core
NBC = 2                     # b-chunks per core
BW = BC // NBC              # 512 (one PSUM bank / max fp32 moving free dim)
KT = 128
NKP = 157                   # K-tiles over G (156 full + one K=32 tail)
GP = NKP * KT               # 20096 (zero-padded from 20000)
KG = 4                      # max k-tiles per DMA group
# ramp-up schedule: small first chunks so the PE starts ~10us sooner
_sizes = [1, 1, 2] + [KG] * ((NKP - 4) // KG) + (
    [NKP - 4 - KG * ((NKP - 4) // KG)] if (NKP - 4) % KG else []
)
KGROUPS = []
_k = 0
for _s in _sizes:
    KGROUPS.append((_k, _s))
    _k += _s
assert _k == NKP
PA = P * A                  # 8192 flattened head outputs
NM3 = PA // 128             # 64 pa-chunks
NM1 = H1 // 128             # 4
NK2 = H1 // 128             # 4
NM2 = H2 // 128             # 2
NK3 = H2 // 128             # 2

_CACHE = {}


def _build(mode: str):
    """Build + compile the per-core Bass graph. mode: 'f32r' | 'f32' | 'bf16'."""
    import concourse.bacc as bacc
    import concourse.tile as tile
    import concourse.mybir as mybir

    FP = mybir.dt.float32
    # ST: storage dtype of streamed/intermediate SBUF tiles (and big DRAM ins)
    ST = mybir.dt.bfloat16 if mode.startswith("bf16") else (
        mybir.dt.float32r if mode == "f32r" else mybir.dt.float32
    )
    DEEP = 6 if mode in ("bf16x", "bf16y") else 4
    PS3B = 5 if mode == "bf16y" else 4

    def mc(ap):
        return ap

    nc = bacc.Bacc(
        "TRN2", target_bir_lowering=False, debug=False, num_devices=NCORES
    )

    # k-tile-interleaved layouts: col block ki holds k-tile ki
    # xTi[p, ki*BC + j]  = x_core[j, ki*128 + p]   (zero-padded past G)
    # w1ti[p, ki*H1 + h] = W1[h, ki*128 + p]
    xT_d = nc.dram_tensor("xTi", [128, NKP * BC], ST, kind="ExternalInput")
    w1t_d = nc.dram_tensor("w1ti", [128, NKP * H1], ST, kind="ExternalInput")
    b1r_d = nc.dram_tensor("b1r", [128, NM1], FP, kind="ExternalInput")
    w2t_d = nc.dram_tensor("w2t", [H1, H2], ST, kind="ExternalInput")
    b2r_d = nc.dram_tensor("b2r", [128, NM2], FP, kind="ExternalInput")
    w3f_d = nc.dram_tensor("w3f", [H2, PA], ST, kind="ExternalInput")
    b3r_d = nc.dram_tensor("b3r", [128, NM3], FP, kind="ExternalInput")
    w4bd_d = nc.dram_tensor("w4bd", [PA, P], ST, kind="ExternalInput")
    b4r_d = nc.dram_tensor("b4r", [128, 1], FP, kind="ExternalInput")
    out_d = nc.dram_tensor("out", [P, BC], FP, kind="ExternalOutput")

    Relu = mybir.ActivationFunctionType.Relu

    with tile.TileContext(nc) as tc:
        with (
            tc.tile_pool(name="const", bufs=1) as const,
            tc.tile_pool(name="h1", bufs=1) as h1pool,
            tc.tile_pool(name="h2", bufs=1) as h2pool,
            tc.tile_pool(name="osb", bufs=1) as opool,
            tc.tile_pool(name="xp", bufs=6 if mode.startswith("bf16") else 4) as xpool,
            tc.tile_pool(name="w1p", bufs=6 if mode.startswith("bf16") else 4) as w1pool,
        ):
            b1t = const.tile([128, NM1], FP)
            nc.scalar.dma_start(b1t[:], b1r_d[:])
            b2t = const.tile([128, NM2], FP)
            nc.scalar.dma_start(b2t[:], b2r_d[:])
            b3t = const.tile([128, NM3], FP)
            nc.scalar.dma_start(b3t[:], b3r_d[:])
            b4t = const.tile([128, 1], FP)
            nc.scalar.dma_start(b4t[:], b4r_d[:])

            # h1T as [128, m(4) x b(1024)]; col m*BC + j holds h1[m*128+p, j]
            h1a = h1pool.tile([128, NM1 * BC], ST)
            h2a = h2pool.tile([128, NM2 * BC], ST)
            outsb = opool.tile([128, BC], FP)

            # ---- L1: h1T = relu(W1T.T @ xT + b1), K over G ----
            with tc.tile_pool(name="ps1", bufs=1, space="PSUM") as ps1:
                pst = [
                    ps1.tile([128, BW], FP, name=f"pst{i}", tag=f"pst{i}")
                    for i in range(NM1 * NBC)
                ]
                for (k0, gsz) in KGROUPS:
                    wt = w1pool.tile([128, KG * H1], ST)
                    nc.sync.dma_start(
                        wt[:, : gsz * H1], w1t_d[:, k0 * H1 : (k0 + gsz) * H1]
                    )
                    xt = xpool.tile([128, KG * BC], ST)
                    nc.sync.dma_start(
                        xt[:, : gsz * BC], xT_d[:, k0 * BC : (k0 + gsz) * BC]
                    )
                    # k-contiguous per PSUM bank: bank switches every gsz MMs
                    # (not every MM) to avoid psum-queue depth-cycling stalls
                    for m in range(NM1):
                        for b in range(NBC):
                            for kk in range(gsz):
                                ki = k0 + kk
                                nc.tensor.matmul(
                                    pst[m * NBC + b][:, :],
                                    mc(wt[:, kk * H1 + m * 128 : kk * H1 + (m + 1) * 128]),
                                    mc(xt[:, kk * BC + b * BW : kk * BC + (b + 1) * BW]),
                                    start=(ki == 0),
                                    stop=(ki == NKP - 1),
                                )
                for m in range(NM1):
                    for b in range(NBC):
                        c0 = m * BC + b * BW
                        if (m * NBC + b) % 2 == 0:
                            nc.scalar.activation(
                                h1a[:, c0 : c0 + BW],
                                pst[m * NBC + b][:, :],
                                Relu,
                                bias=b1t[:, m : m + 1],
                            )
                        else:
                            nc.vector.tensor_scalar(
                                h1a[:, c0 : c0 + BW],
                                pst[m * NBC + b][:, :],
                                b1t[:, m : m + 1],
                                0.0,
                                op0=mybir.AluOpType.add,
                                op1=mybir.AluOpType.max,
                            )

            # ---- L2: h2T = relu(W2T.T @ h1T + b2), K over H1 ----
            with (
                tc.tile_pool(name="w2p", bufs=NK2) as w2pool,
                tc.tile_pool(name="ps2", bufs=1, space="PSUM") as ps2,
            ):
                pst2 = [
                    ps2.tile([128, BW], FP, name=f"pst2_{i}", tag=f"pst2_{i}")
                    for i in range(NM2 * NBC)
                ]
                for ki in range(NK2):
                    w2t = w2pool.tile([128, H2], ST)
                    nc.scalar.dma_start(w2t[:], w2t_d[ki * 128 : (ki + 1) * 128, :])
                    for m in range(NM2):
                        for b in range(NBC):
                            nc.tensor.matmul(
                                pst2[m * NBC + b][:, :],
                                mc(w2t[:, m * 128 : (m + 1) * 128]),
                                mc(h1a[:, ki * BC + b * BW : ki * BC + b * BW + BW]),
                                start=(ki == 0),
                                stop=(ki == NK2 - 1),
                            )
                for m in range(NM2):
                    for b in range(NBC):
                        c0 = m * BC + b * BW
                        if (m * NBC + b) % 2 == 0:
                            nc.scalar.activation(
                                h2a[:, c0 : c0 + BW],
                                pst2[m * NBC + b][:, :],
                                Relu,
                                bias=b2t[:, m : m + 1],
                            )
                        else:
                            nc.vector.tensor_scalar(
                                h2a[:, c0 : c0 + BW],
                                pst2[m * NBC + b][:, :],
                                b2t[:, m : m + 1],
                                0.0,
                                op0=mybir.AluOpType.add,
                                op1=mybir.AluOpType.max,
                            )

            # ---- L3+L4: aT chunks then block-diag W4 reduction ----
            with (
                tc.tile_pool(name="w3p", bufs=DEEP) as w3pool,
                tc.tile_pool(name="w4p", bufs=DEEP) as w4pool,
                tc.tile_pool(name="ap", bufs=DEEP) as apool,
                tc.tile_pool(name="ps4", bufs=1, space="PSUM") as ps4,
                tc.tile_pool(name="ps3", bufs=PS3B, space="PSUM") as ps3pool,
            ):
                po = [ps4.tile([128, BW], FP, name=f"po{i}", tag=f"po{i}") for i in range(NBC)]
                # software-pipelined: L4 accumulation for step mi-1 is emitted
                # between step mi's L3 matmuls so the PSUM->SBUF eviction
                # latency never blocks the PE stream.
                pend = []  # (mi, b, w4t, at) awaiting their L4 matmul

                def flush_l4():
                    for (pmi, pb, pw4t, pat) in pend:
                        nc.tensor.matmul(
                            po[pb][:, :],
                            mc(pw4t[:, :]),
                            mc(pat[:, :]),
                            start=(pmi == 0),
                            stop=(pmi == NM3 - 1),
                        )
                    pend.clear()

                for mi in range(NM3):
                    w3t = w3pool.tile([128, H2], ST)
                    for k in range(NK3):
                        nc.sync.dma_start(
                            w3t[:, k * 128 : (k + 1) * 128],
                            w3f_d[k * 128 : (k + 1) * 128, mi * 128 : (mi + 1) * 128],
                        )
                    w4t = w4pool.tile([128, 128], ST)
                    nc.sync.dma_start(w4t[:], w4bd_d[mi * 128 : (mi + 1) * 128, :])
                    mypend = []
                    for b in range(NBC):
                        ps3 = ps3pool.tile([128, BW], FP)
                        for k in range(NK3):
                            nc.tensor.matmul(
                                ps3[:, :],
                                mc(w3t[:, k * 128 : (k + 1) * 128]),
                                mc(h2a[:, k * BC + b * BW : k * BC + b * BW + BW]),
                                start=(k == 0),
                                stop=(k == NK3 - 1),
                            )
                        if b == NBC - 1:
                            flush_l4()  # previous mi's L4 after this mi's L3 MMs
                        at = apool.tile([128, BW], ST)
                        if (mi * NBC + b) % 5 < 3:
                            nc.scalar.activation(
                                at[:, :], ps3[:, :], Relu, bias=b3t[:, mi : mi + 1]
                            )
                        else:
                            # relu(x + b3) on VectorE: (x add b3) max 0
                            nc.vector.tensor_scalar(
                                at[:, :],
                                ps3[:, :],
                                b3t[:, mi : mi + 1],
                                0.0,
                                op0=mybir.AluOpType.add,
                                op1=mybir.AluOpType.max,
                            )
                        mypend.append((mi, b, w4t, at))
                    pend.extend(mypend)
                flush_l4()
                for b in range(NBC):
                    nc.vector.tensor_scalar_add(
                        outsb[:, b * BW : (b + 1) * BW], po[b][:, :], b4t[:, 0:1]
                    )
            nc.sync.dma_start(out_d[:, :], outsb[:, :])

    nc.compile()
    return nc


def _get_nc(mode: str):
    if mode not in _CACHE:
        _CACHE[mode] = _build(mode)
    return _CACHE[mode]


def _interleave_k(mat_gp: np.ndarray) -> np.ndarray:
    """[GP, F] -> [128, NKP*F] with col block ki = k-tile ki."""
    f = mat_gp.shape[1]
    return np.ascontiguousarray(
        mat_gp.reshape(NKP, 128, f).transpose(1, 0, 2).reshape(128, NKP * f)
    )


def _prep_inputs(x, W1, b1, W2, b2, W3, b3, W4, b4, mode="f32r"):
    f = np.float32
    if mode.startswith("bf16"):
        import ml_dtypes

        st = np.dtype(ml_dtypes.bfloat16)
    else:
        st = np.dtype(np.float32)
    ac = np.ascontiguousarray

    def cst(a):
        return a if a.dtype == st else a.astype(st)

    x = np.asarray(x, f)
    xTp = np.zeros((GP, B), st)
    np.copyto(xTp[:G], cst(x.T))                               # [GP, B]
    w1tp = np.zeros((GP, H1), st)
    np.copyto(w1tp[:G], cst(np.asarray(W1, f).T))
    w1ti = _interleave_k(w1tp)                                 # [128, NKP*H1]
    b1r = ac(np.asarray(b1, f).reshape(NM1, 128).T)            # [128, 4]
    w2t = ac(cst(np.asarray(W2, f).T))                         # [H1, H2]
    b2r = ac(np.asarray(b2, f).reshape(NM2, 128).T)            # [128, 2]
    w3f = ac(cst(np.asarray(W3, f).transpose(1, 0, 2).reshape(H2, PA)))
    b3r = ac(np.asarray(b3, f).reshape(PA).reshape(NM3, 128).T)  # [128, 64]
    w4bd = np.zeros((PA, P), st)
    w4bd[np.arange(PA), np.arange(PA) // A] = cst(np.asarray(W4, f).reshape(PA))
    b4r = ac(np.asarray(b4, f).reshape(128, 1))

    shared = {
        "w1ti": w1ti, "b1r": b1r, "w2t": w2t, "b2r": b2r,
        "w3f": w3f, "b3r": b3r, "w4bd": w4bd, "b4r": b4r,
    }
    in_maps = []
    for c in range(NCORES):
        m = {"xTi": _interleave_k(xTp[:, c * BC : (c + 1) * BC])}
        m.update(shared)
        in_maps.append(m)
    return in_maps


def run_with_results(inputs: dict, trace: bool = False, mode: str | None = None):
    """Returns (full_output [B, P] float32, BassKernelResults)."""
    from concourse.bass_utils import run_bass_kernel_spmd

    if mode is None:
        mode = os.environ.get("CTP_MODE", "bf16x")
    nc = _get_nc(mode)
    in_maps = _prep_inputs(**inputs, mode=mode)
    res = run_bass_kernel_spmd(
        nc, in_maps, core_ids=list(range(NCORES)), trace=trace
    )
    out = np.empty((B, P), np.float32)
    for c in range(NCORES):
        out[c * BC : (c + 1) * BC, :] = res.results[c]["out"].T
    return out, res


def kernel(**inputs) -> np.ndarray:
    out, _ = run_with_results(inputs, trace=False)
    return out
